# revision 1
# baseline (speedup 1.0000x reference)
"""Multi-head self-attention (qk-l2-normalized) TRN2 Bass kernel.

Reference computation (T=4096, D=2048, H=16, HD=128):
    qkv = x @ W_qkv ; q,k,v = split(qkv)
    per head: qn = l2norm(q), kn = l2norm(k)
              attn = softmax(qn @ kn.T * HD**-0.5 + mask)
              o = attn @ v
    out = concat_heads(o) @ W_out

Sharding: tensor-parallel over heads.  Core c owns heads {2c, 2c+1}:
W_qkv column slices + W_out row slices.  Each core computes a partial
(T, D) output; the host sums the 8 partials (the "all-reduce").

Device algorithm per core (everything transpose-free):
  - host supplies xT = x.T (fp16).  QT/KT computed directly transposed
    (d on partitions) via lhsT=W-slices, rhs=xT.  V computed in natural
    layout (token on partitions) via lhsT=xT, rhs=Wv.
  - row norms of q/k via DVE square + ones-matmul (cross-partition sum),
    sqrt on ACT, reciprocal on DVE; the HD**-0.5 scale is folded into rk.
  - the 1/|q|, 1/|k| row scalings are applied as rank-1 broadcast
    multiplies (ones ⊗ row outer-product on PE, then DVE multiply).
  - S^T = KnT.T @ QnT  (j on partitions, t free) -> exp on ACT (fp16)
    -> flash-style: attn@v accumulates OT in PSUM over j-chunks while
    DVE accumulates the softmax denominator; final column scale by 1/Z.
  - out partial = OT.T-free matmul with lhsT=OT slices, rhs=W_out rows.
"""

import os
import sys

import numpy as np

if "/opt/trn_rl_repo" not in sys.path:
    sys.path.insert(0, "/opt/trn_rl_repo")

T, D, H, NCORES = 4096, 2048, 16, 8
HD = D // H            # 128 head dim
HPC = H // NCORES      # 2 heads per core
DH = HPC * HD          # 256 local head columns
EPS = 1e-12
SCALE = HD ** -0.5

_PROG_CACHE = {}


def _split_drain_tc(nc, tile):
    """TileContext that never emits more than one semaphore wait per inst.

    This walrus build encodes only a single sync wait per instruction
    ("Too many sync wait commands" otherwise).  Two fixes:
    - interior instructions: after Tile's sem assignment, excess waits are
      moved onto same-engine InstNoOps inserted immediately before the
      instruction (engines execute their stream in order, so semantics are
      identical);
    - the kernel-tail drain: emit one wait-carrying SP nop per logical proc
      instead of attaching the whole global clock to the drain.
    """
    import bass_rust
    import concourse.mybir as mybir
    from concourse.vector_clock import ScopedClock, VectorClock

    MAXW = 1

    class SplitWaitTC(tile.TileContext):
        def _lower_ordered_insts(self, ordered):
            for bb_name, insts in ordered.items():
                new = []
                for inst in insts:
                    si = None
                    try:
                        si = inst.sync_info
                    except Exception:
                        pass
                    if si is not None and len(si.on_wait) > MAXW:
                        waits = list(si.on_wait)
                        keep, extra = waits[-MAXW:], waits[:-MAXW]
                        for i, w in enumerate(extra):
                            new.append(mybir.InstNoOp(
                                name=f"{inst.name}ws{i}",
                                engine=inst.engine,
                                bass_nofuse=True,
                                sync_info=bass_rust.SyncInfo(
                                    on_wait=[w], on_update=[]),
                            ))
                        inst.sync_info = bass_rust.SyncInfo(
                            on_wait=keep, on_update=list(si.on_update))
                    new.append(inst)
                ordered[bb_name] = new
            return super()._lower_ordered_insts(ordered)

        def _drain_and_barrier(self, tick_clock, wait_clock):
            ticks = eval(
                str(tick_clock.global_clock).replace("VectorClock(", "").rstrip(")"))
            for p, tk in enumerate(ticks):
                if tk > 0:
                    sub = VectorClock()
                    sub.require_at_least(p, tk)
                    nop = self.nc.sync.nop(nofuse=True)
                    wait_clock.add_sem_waits(nop.ins, ScopedClock({None: sub}))
            self.nc.sync.drain()
            self.nc.all_engine_barrier()
            assert self.sems is not None
            popped = self.nc._tile_sem_poison_stack.pop()
            assert popped is self._sem_poison
            self.nc.clear_and_free_semaphores(list(self.sems.allocated().values()))
            self.nc.all_engine_barrier()

    return SplitWaitTC(nc)


def build_program(t=T, with_mask=False):
    """Build the single-core Bass/Tile program (same program on all cores)."""
    import concourse.bass as bass
    import concourse.mybir as mybir
    import concourse.tile as tile

    dt = mybir.dt
    f32, f16 = dt.float32, dt.float16
    AF = mybir.ActivationFunctionType

    KC = D // 128          # 16 contraction chunks for projections
    TTS = 512              # token tile size (free dim of most matmuls)
    NTT = t // TTS         # number of token tiles
    NJC = t // 128         # number of key chunks
    NST = TTS // 128       # 128-token subtiles per token tile

    nc = bass.Bass(trn_type="TRN2")
    xT_d = nc.dram_tensor("xT", (D, t), f16, kind="ExternalInput")
    wq_d = nc.dram_tensor("wq", (D, DH), f16, kind="ExternalInput")
    wk_d = nc.dram_tensor("wk", (D, DH), f16, kind="ExternalInput")
    wv_d = nc.dram_tensor("wv", (D, DH), f16, kind="ExternalInput")
    wo_d = nc.dram_tensor("wo", (DH, D), f16, kind="ExternalInput")
    if with_mask:
        mT_d = nc.dram_tensor("maskT", (t, t), f16, kind="ExternalInput")
    y_d = nc.dram_tensor("y", (t, D), f32, kind="ExternalOutput")

    xT_t = xT_d[:].rearrange("(kc p) t -> p kc t", p=128)   # (128, KC, t)

    with _split_drain_tc(nc, tile) as tc:
        with (
            tc.tile_pool(name="consts", bufs=1) as cpool,
            tc.tile_pool(name="wts", bufs=1) as wpool,
            tc.tile_pool(name="big", bufs=1) as bigpool,
            tc.tile_pool(name="xcs", bufs=2) as xpool,
            tc.tile_pool(name="work", bufs=2) as work,
            tc.tile_pool(name="rows", bufs=3) as rows,
            tc.tile_pool(name="ps", bufs=1, space="PSUM") as psum,
        ):
            # PSUM budget (8 banks):
            #   mm2: (128,1024) 2-bank x2 = 4  [proj pairs, S^T pairs, outproj pairs]
            #   p1:  (128,512)  1-bank x2 = 2  [V proj, OT accumulator]
            #   aux: (128,512)  1-bank x2 = 2  [normsq, rq bcast, Z, rs bcast]

            # ---- constants -------------------------------------------------
            ones_col = cpool.tile([1, 128], f16)    # lhsT for row->(128,·) bcast
            nc.vector.memset(ones_col[:], 1.0)
            ones_red = cpool.tile([128, 1], f16)    # lhsT for partition-sum
            nc.vector.memset(ones_red[:], 1.0)
            ln_scale_c = cpool.tile([1, 1], f32)    # bias: ln(SCALE) for rk
            nc.vector.memset(ln_scale_c[:], float(np.log(SCALE)))

            # ---- persistent activations -----------------------------------
            # QnT/KnT: (128=d, h, t) normalized fp16.  V: (128=j, NJC, DH).
            qnt = bigpool.tile([128, HPC, t], f16, name="qnt")
            knt = bigpool.tile([128, HPC, t], f16, name="knt")
            vsb = bigpool.tile([128, NJC, DH], f16, name="vsb")

            # ---- stage weights resident in SBUF ---------------------------
            # (first x chunk is prefetched before the weights so the first
            #  projection matmuls start as early as possible)
            xc0 = xpool.tile([128, KC, TTS], f16, tag="xc", bufs=3)
            for kh in range(4):
                nc.sync.dma_start(xc0[:, kh * 4:(kh + 1) * 4, :],
                                  xT_t[:, kh * 4:(kh + 1) * 4, 0:TTS])
            wq_sb = wpool.tile([128, KC, DH], f16)
            nc.sync.dma_start(wq_sb[:], wq_d[:].rearrange("(kc p) m -> p kc m", p=128))
            wk_sb = wpool.tile([128, KC, DH], f16)
            nc.sync.dma_start(wk_sb[:], wk_d[:].rearrange("(kc p) m -> p kc m", p=128))
            wv_sb = wpool.tile([128, KC, DH], f16)
            nc.sync.dma_start(wv_sb[:], wv_d[:].rearrange("(kc p) m -> p kc m", p=128))
            wo_sb = wpool.tile([128, HPC, D], f16)
            nc.sync.dma_start(wo_sb[:], wo_d[:].rearrange("(h p) n -> p h n", p=128))

            # ================= Phase 1: QKV projections ====================
            for tt in range(NTT):
                tsl = slice(tt * TTS, (tt + 1) * TTS)
                if tt == 0:
                    xc = xc0
                else:
                    xc = xpool.tile([128, KC, TTS], f16, tag="xc", bufs=3,
                                    name="xc")
                    nc.sync.dma_start(xc[:], xT_t[:, :, tsl])

                # q-pair then k-pair: both heads' projections batched 2-bank
                for (mat, w_sb, dst, is_k) in (
                    ("q", wq_sb, qnt, False),
                    ("k", wk_sb, knt, True),
                ):
                    pj = psum.tile([128, 2 * TTS], f32, name=f"pj_{mat}_{tt}",
                                   tag="mm2", bufs=2)
                    for hh in range(HPC):
                        for kc in range(KC):
                            nc.tensor.matmul(
                                pj[:, hh * TTS:(hh + 1) * TTS],
                                w_sb[:, kc, hh * 128:(hh + 1) * 128],
                                xc[:, kc, :], start=(kc == 0),
                                stop=(kc == KC - 1))
                    # raw (d, 2*t) pair to fp16 (frees the 2-bank psum)
                    qts = work.tile([128, 2 * TTS], f16, tag="qts", bufs=2)
                    nc.vector.tensor_copy(qts[:], pj[:])
                    sq = work.tile([128, 2 * TTS], f16, tag="sq", bufs=2)
                    nc.vector.tensor_mul(sq[:], qts[:], qts[:])
                    # 1/||row|| entirely on ACT (natural_log_exp set, which
                    # also holds exp/copy -> a single table set kernel-wide):
                    # s/sqrt(x) = Exp(-0.5*Ln(x) + ln(s)).  s=SCALE for k
                    # folds the attention scale in; s=1 for q.
                    ln_bias = ln_scale_c[:] if is_k else 0.0
                    for hh in range(HPC):
                        hsl = slice(hh * TTS, (hh + 1) * TTS)
                        nsq = psum.tile([1, TTS], f32, name=f"nsq_{mat}_{tt}_{hh}",
                                        tag="aux", bufs=2)
                        nc.tensor.matmul(nsq[:], ones_red[:], sq[:, hsl])
                        lnr = rows.tile([1, TTS], f32, tag="lnr", bufs=3)
                        nc.scalar.activation(lnr[:], nsq[:], AF.Ln)
                        rq16 = rows.tile([1, TTS], f16, tag="rq16", bufs=3)
                        nc.scalar.activation(rq16[:], lnr[:], AF.Exp,
                                             scale=-0.5, bias=ln_bias)
                        # broadcast row across partitions: ones_col ⊗ rq16
                        rqb = psum.tile([128, TTS], f32, name=f"rqb_{mat}_{tt}_{hh}",
                                        tag="aux", bufs=2)
                        nc.tensor.matmul(rqb[:], ones_col[:], rq16[:])
                        nc.vector.tensor_mul(dst[:, hh, tsl], qts[:, hsl], rqb[:])

                # V for both heads, natural layout; two 128-token subtiles
                # share one 1-bank psum tile (two halves)
                for sp in range(NST // 2):
                    vp = psum.tile([128, 2 * DH], f32, name=f"vp_{tt}_{sp}",
                                   tag="p1", bufs=2)
                    for half in range(2):
                        st = sp * 2 + half
                        for kc in range(KC):
                            nc.tensor.matmul(
                                vp[:, half * DH:(half + 1) * DH],
                                xc[:, kc, st * 128:(st + 1) * 128],
                                wv_sb[:, kc, :], start=(kc == 0),
                                stop=(kc == KC - 1))
                    jidx = tt * NST + sp * 2
                    nc.vector.tensor_copy(vsb[:, jidx:jidx + 2, :], vp[:])

            # ============ Phase 2+3: attention + output projection =========
            NJQ = NJC // 4          # j-quads (4 chunks of 128 keys)
            for tt in range(NTT):
                tsl = slice(tt * TTS, (tt + 1) * TTS)
                ot_sb = [None, None]
                for h in range(HPC):
                    ot = psum.tile([128, TTS], f32, name=f"ot_{tt}_{h}",
                                   tag="p1", bufs=2)
                    acc = work.tile([128, TTS], f32, tag="acc", bufs=3)
                    NJP = NJC // 2           # 2-chunk pairs
                    e_tiles = {}

                    def st_pair(jp):
                        stp = psum.tile([128, 2 * TTS], f32,
                                        name=f"st_{tt}_{h}_{jp}",
                                        tag="mm2", bufs=2)
                        for jh in range(2):
                            jc = jp * 2 + jh
                            nc.tensor.matmul(
                                stp[:, jh * TTS:(jh + 1) * TTS],
                                knt[:, h, jc * 128:(jc + 1) * 128],
                                qnt[:, h, tsl], start=True, stop=True)
                        return stp

                    def exp_pair(jp, stp):
                        jq, half = jp // 2, jp % 2
                        if half == 0:
                            e_tiles[jq] = work.tile([128, 4 * TTS], f16,
                                                    tag="e", bufs=3, name="e")
                        e = e_tiles[jq]
                        esl = slice(half * 2 * TTS, (half + 1) * 2 * TTS)
                        if with_mask:
                            jc0 = jp * 2
                            mc = work.tile([128, 2, TTS], f16, tag="mc", bufs=3)
                            nc.sync.dma_start(
                                mc[:],
                                mT_d[:].rearrange("(c p) t -> p c t", p=128)
                                [:, jc0:jc0 + 2, tsl])
                            sm = work.tile([128, 2 * TTS], f32, tag="sm", bufs=3)
                            nc.vector.tensor_add(sm[:], stp[:], mc[:])
                            nc.scalar.activation(e[:, esl], sm[:], AF.Exp)
                        else:
                            nc.scalar.activation(e[:, esl], stp[:], AF.Exp)

                    def ot_pair(jp):
                        e = e_tiles[jp // 2]
                        for jh in range(2):
                            jc = jp * 2 + jh
                            lsl = slice((jp % 2 * 2 + jh) * TTS,
                                        (jp % 2 * 2 + jh + 1) * TTS)
                            nc.tensor.matmul(
                                ot[:], vsb[:, jc, h * 128:(h + 1) * 128],
                                e[:, lsl], start=(jc == 0),
                                stop=(jc == NJC - 1), skip_group_check=True)

                    def tree(jq):
                        # fp16 pair tree + f32 accumulate (exact in f32)
                        e = e_tiles.pop(jq)
                        t0 = work.tile([128, TTS], f16, tag="t0", bufs=3)
                        nc.vector.tensor_add(t0[:], e[:, 0:TTS],
                                             e[:, TTS:2 * TTS])
                        t1 = work.tile([128, TTS], f16, tag="t1", bufs=3)
                        nc.vector.tensor_add(t1[:], e[:, 2 * TTS:3 * TTS],
                                             e[:, 3 * TTS:4 * TTS])
                        if jq == 0:
                            nc.vector.tensor_add(acc[:], t0[:], t1[:])
                        else:
                            t2 = work.tile([128, TTS], f16, tag="t2", bufs=3)
                            nc.vector.tensor_add(t2[:], t0[:], t1[:])
                            nc.vector.tensor_add(acc[:], acc[:], t2[:])

                    # software pipeline, depth 2: OT(jp) issues only after
                    # exp(jp) AND two newer ST pairs, so the PE never stalls
                    # on the ACT exp latency.
                    stps = [st_pair(0), st_pair(1)]
                    for jp in range(NJP):
                        exp_pair(jp, stps[jp % 2])
                        if jp + 2 < NJP:
                            stps[jp % 2] = st_pair(jp + 2)
                        ot_pair(jp)
                        if jp % 2 == 1:
                            tree(jp // 2)
                    # denominator -> 1/Z = Exp(-Ln(Z)) -> broadcast -> scale
                    acch = work.tile([128, TTS], f16, tag="acch", bufs=2)
                    nc.vector.tensor_copy(acch[:], acc[:])
                    z = psum.tile([1, TTS], f32, name=f"z_{tt}_{h}",
                                  tag="aux", bufs=2)
                    nc.tensor.matmul(z[:], ones_red[:], acch[:])
                    lnz = rows.tile([1, TTS], f32, tag="lnz", bufs=3)
                    nc.scalar.activation(lnz[:], z[:], AF.Ln)
                    rs16 = rows.tile([1, TTS], f16, tag="rs16", bufs=3)
                    nc.scalar.activation(rs16[:], lnz[:], AF.Exp, scale=-1.0)
                    rsb = psum.tile([128, TTS], f32, name=f"rsb_{tt}_{h}",
                                    tag="aux", bufs=2)
                    nc.tensor.matmul(rsb[:], ones_col[:], rs16[:])
                    rsbs = work.tile([128, TTS], f32, tag="rsbs", bufs=2)
                    nc.vector.tensor_copy(rsbs[:], rsb[:])
                    osb = work.tile([128, TTS], f16, tag=f"osb{h}", bufs=2)
                    nc.vector.tensor_mul(osb[:], ot[:], rsbs[:])
                    ot_sb[h] = osb

                # output projection: single-bank psum tiles in the p1 tag so
                # this overlaps the next tile's attention (mm2) instead of
                # contending with it.
                for st in range(NST):
                    for ng in range(D // 1024):
                        # two n-tiles per group, h outermost: the stationary
                        # operand (ot slice) is reused across both matmuls
                        ops = []
                        for half in range(2):
                            nt = ng * 2 + half
                            ops.append(psum.tile(
                                [128, 512], f32, name=f"op_{tt}_{st}_{nt}",
                                tag="p1", bufs=2))
                        for h in range(HPC):
                            for half in range(2):
                                nt = ng * 2 + half
                                nc.tensor.matmul(
                                    ops[half][:],
                                    ot_sb[h][:, st * 128:(st + 1) * 128],
                                    wo_sb[:, h, nt * 512:(nt + 1) * 512],
                                    start=(h == 0), stop=(h == HPC - 1),
                                    skip_group_check=True)
                        for half in range(2):
                            nt = ng * 2 + half
                            oc = work.tile([128, 512], f32, tag="oc", bufs=4)
                            nc.vector.tensor_copy(oc[:], ops[half][:])
                            nc.sync.dma_start(
                                y_d[tt * TTS + st * 128:
                                    tt * TTS + (st + 1) * 128,
                                    nt * 512:(nt + 1) * 512], oc[:])

    return nc


def _get_program(t=T, with_mask=False):
    key = (t, with_mask)
    if key not in _PROG_CACHE:
        _PROG_CACHE[key] = build_program(t, with_mask)
    return _PROG_CACHE[key]


def _make_in_maps(x, attn_mask, W_qkv, W_out, use_mask):
    t = x.shape[0]
    xT16 = np.ascontiguousarray(x.T).astype(np.float16)
    wq_f = W_qkv[:, 0 * D:1 * D]
    wk_f = W_qkv[:, 1 * D:2 * D]
    wv_f = W_qkv[:, 2 * D:3 * D]
    maskT = None
    if use_mask:
        maskT = np.ascontiguousarray(attn_mask.T).astype(np.float16)
    in_maps = []
    for c in range(NCORES):
        cs = slice(c * DH, (c + 1) * DH)
        m = {
            "xT": xT16,
            "wq": np.ascontiguousarray(wq_f[:, cs]).astype(np.float16),
            "wk": np.ascontiguousarray(wk_f[:, cs]).astype(np.float16),
            "wv": np.ascontiguousarray(wv_f[:, cs]).astype(np.float16),
            "wo": np.ascontiguousarray(W_out[cs, :]).astype(np.float16),
        }
        if use_mask:
            m["maskT"] = maskT
        in_maps.append(m)
    return in_maps


def run_raw(x, attn_mask, W_qkv, W_out, trace=False, **kwargs):
    """Run the SPMD kernel; returns (full_output, BassKernelResults)."""
    from concourse.bass_utils import run_bass_kernel_spmd

    x = np.asarray(x, dtype=np.float32)
    attn_mask = np.asarray(attn_mask, dtype=np.float32)
    W_qkv = np.asarray(W_qkv, dtype=np.float32)
    W_out = np.asarray(W_out, dtype=np.float32)

    use_mask = bool(np.any(attn_mask))
    nc = _get_program(x.shape[0], use_mask)
    in_maps = _make_in_maps(x, attn_mask, W_qkv, W_out, use_mask)
    res = run_bass_kernel_spmd(nc, in_maps, core_ids=list(range(NCORES)),
                               trace=trace, **kwargs)
    out = np.zeros((x.shape[0], D), np.float32)
    for r in res.results:
        out += r["y"]
    return out, res


def kernel(x, attn_mask, W_qkv, W_out):
    out, _ = run_raw(x, attn_mask, W_qkv, W_out)
    return out

